# revision 32
# baseline (speedup 1.0000x reference)
"""Bahdanau attention with coverage — Trainium2 Bass kernel, 8-core data parallel.

Shards batch B=16 across 8 NeuronCores (2 batches/core). Per core:
  enc_feat = enc @ W_h.T  (bf16 matmul, PE; enc tiles transposed on PE)
  + dec_feat + cov*w_c    (folded into the PSUM accumulation as a K=2 rank-2 matmul)
  tanh (ACT, reads PSUM) -> dot with v (gpsimd mult + DVE reduce) -> scores
  softmax over S=4096 (DVE free-reduce + gpsimd partition_all_reduce), mask+renorm
  context = attn @ enc    (PE, from resident bf16 copy of enc in SBUF)
"""

import os
import sys

import numpy as np

try:
    import concourse.bass as bass  # noqa: F401
except ImportError:
    sys.path.insert(0, "/opt/trn_rl_repo")

import concourse.bass as bass
import concourse.mybir as mybir
import concourse.tile as tile
from concourse import bacc, bass_isa
from concourse.bass_utils import run_bass_kernel_spmd
from concourse.masks import make_identity

P = 128
B = 16
S = 4096
D = 1024
N_CORES = 8
BPC = B // N_CORES        # batches per core
S_TILES = S // P          # 32
DC = D // P               # 8 d-chunks
EH = 512                  # psum half of e dim
F32 = mybir.dt.float32
BF16 = mybir.dt.bfloat16

_CACHE = {}


def _bcast_ap(dram_ap, n_part):
    """AP that replicates a 1-D dram tensor across n_part partitions."""
    return bass.AP(
        tensor=dram_ap.tensor,
        offset=dram_ap.offset,
        ap=[[0, n_part]] + list(dram_ap.ap),
    )


def _build():
    nc = bacc.Bacc("TRN2", target_bir_lowering=False, debug=False,
                   num_devices=N_CORES)

    enc = nc.dram_tensor("encoder_output", [BPC, S, D], F32, kind="ExternalInput")
    dec = nc.dram_tensor("decoder_hidden", [BPC, 1, D], F32, kind="ExternalInput")
    mask = nc.dram_tensor("mask", [BPC, S], F32, kind="ExternalInput")
    cov = nc.dram_tensor("coverage", [BPC, S], F32, kind="ExternalInput")
    W_h = nc.dram_tensor("W_h", [D, D], F32, kind="ExternalInput")
    W_s = nc.dram_tensor("W_s", [D, D], F32, kind="ExternalInput")
    b_s = nc.dram_tensor("b_s", [D], F32, kind="ExternalInput")
    w_c = nc.dram_tensor("w_c", [D], F32, kind="ExternalInput")
    v = nc.dram_tensor("v", [D], F32, kind="ExternalInput")
    ctx_o = nc.dram_tensor("context_out", [BPC, D], F32, kind="ExternalOutput")
    attn_o = nc.dram_tensor("attn_out", [BPC, S], F32, kind="ExternalOutput")
    cov_o = nc.dram_tensor("coverage_out", [BPC, S], F32, kind="ExternalOutput")

    AF = mybir.ActivationFunctionType
    ALU = mybir.AluOpType

    with tile.TileContext(nc) as tc:
        with (
            tc.tile_pool(name="singles", bufs=1) as singles,
            tc.tile_pool(name="wt", bufs=1) as wtp,
            tc.tile_pool(name="stream", bufs=4) as stream,
            tc.tile_pool(name="encres", bufs=38) as encresp,
            tc.tile_pool(name="enct", bufs=4) as enctp,
            tc.tile_pool(name="feats", bufs=3) as featsp,
            tc.tile_pool(name="mv", bufs=3) as mvp,
            tc.tile_pool(name="perb", bufs=2) as perb,
            tc.tile_pool(name="small", bufs=2) as small,
            tc.tile_pool(name="psT", bufs=3, space="PSUM") as psT,
            tc.tile_pool(name="psF", bufs=4, space="PSUM") as psF,
            tc.tile_pool(name="psC", bufs=1, space="PSUM") as psC,
        ):
            # ---------------- one-time constants ----------------
            # (small constant loads go on the gpsimd SWDGE queue so the sync
            # HWDGE queue starts on W_h immediately)
            ident = singles.tile([P, P], BF16)
            make_identity(nc, ident)

            v_rep = singles.tile([P, D], F32)
            nc.gpsimd.dma_start(out=v_rep, in_=_bcast_ap(v[:], P))

            b_s_sb = singles.tile([1, D], F32)
            nc.gpsimd.dma_start(out=b_s_sb, in_=b_s[None, :])

            w_c_bf = singles.tile([1, D], BF16)
            nc.gpsimd.dma_start(out=w_c_bf, in_=w_c[None, :])

            # Transposed bf16 weights: WT[di, dc, e] = W[e, dc*128+di]
            def load_transposed(Wdram, tag):
                WT = wtp.tile([P, DC, D], BF16, tag=tag)
                for i in range(DC):  # e-tile
                    wrow = stream.tile([P, D], F32, tag="enc_f32")
                    nc.sync.dma_start(out=wrow, in_=Wdram[i * P:(i + 1) * P, :])
                    wbf = enctp.tile([P, D], BF16, tag="encT")
                    nc.vector.tensor_copy(wbf, wrow)
                    for half in range(2):
                        pt = psT.tile([P, EH], BF16, tag="psT")
                        for j in range(4):
                            dcj = half * 4 + j
                            nc.tensor.transpose(
                                pt[:, j * P:(j + 1) * P],
                                wbf[:, dcj * P:(dcj + 1) * P],
                                ident,
                            )
                        nc.scalar.copy(
                            out=WT[:, half * 4:(half + 1) * 4, i * P:(i + 1) * P],
                            in_=pt.rearrange("p (a b) -> p a b", a=4),
                        )
                return WT

            # W_hT first: phase A's main matmuls depend only on it.
            W_hT = load_transposed(W_h, "W_hT")
            W_sT = load_transposed(W_s, "W_sT")

            # ---- dec rows for both batches (W_sT is transient) ----
            wc_decs = []
            for b in range(BPC):
                dec_sb = small.tile([P, DC], F32, tag="dec_sb")
                nc.sync.dma_start(
                    out=dec_sb, in_=dec[b, 0, :].rearrange("(do di) -> di do", di=P)
                )
                dec_bf = small.tile([P, DC], BF16, tag="dec_bf")
                nc.vector.tensor_copy(dec_bf, dec_sb)

                dec_row_f32 = small.tile([1, D], F32, tag="dec_row")
                for eh in range(2):
                    pd = psF.tile([1, EH], F32, tag="psF")
                    for dc in range(DC):
                        nc.tensor.matmul(
                            pd,
                            lhsT=dec_bf[:, dc:dc + 1],
                            rhs=W_sT[:, dc, eh * EH:(eh + 1) * EH],
                            start=(dc == 0),
                            stop=(dc == DC - 1),
                        )
                    nc.vector.tensor_tensor(
                        dec_row_f32[:, eh * EH:(eh + 1) * EH],
                        pd,
                        b_s_sb[:, eh * EH:(eh + 1) * EH],
                        ALU.add,
                    )
                dec_row_bf = small.tile([1, D], BF16, tag="dec_row_bf")
                nc.vector.tensor_copy(dec_row_bf, dec_row_f32)

                # wc_dec: [2, D] bf16 — row0 = w_c, row1 = dec_feat+b_s
                wc_dec = perb.tile([2, D], BF16, tag="wc_dec")
                nc.sync.dma_start(out=wc_dec[0:1, :], in_=w_c_bf)
                nc.sync.dma_start(out=wc_dec[1:2, :], in_=dec_row_bf)
                wc_decs.append(wc_dec)

            # cov_ones: [2, S] bf16 — row0 = coverage (f32->bf16 via gpsimd
            # converting DMA, per batch), row1 = ones (memset once).
            cov_ones = singles.tile([2, S], BF16, tag="cov_ones")
            nc.gpsimd.memset(cov_ones, 1.0)

            for b in range(BPC):
                wc_dec = wc_decs[b]

                nc.gpsimd.dma_start(out=cov_ones[0:1, :], in_=cov[b, None, :])

                # coverage & mask in [p, t] layout for the small phase
                cov_pt = perb.tile([P, S_TILES], F32, tag="cov_pt")
                nc.sync.dma_start(
                    out=cov_pt, in_=cov[b, :].rearrange("(t p) -> p t", p=P)
                )
                mask_pt = perb.tile([P, S_TILES], F32, tag="mask_pt")
                nc.sync.dma_start(
                    out=mask_pt, in_=mask[b, :].rearrange("(t p) -> p t", p=P)
                )

                scores = perb.tile([P, S_TILES], F32, tag="scores")

                # ---------------- phase A: scores ----------------
                # Software-pipelined: tile t+1's load/cast/transpose is issued
                # before tile t's matmuls so PE never stalls on the
                # transpose -> ACT-copyback -> matmul chain.
                enc_res = []
                encTs = {}

                def stage_tile(t, b=b, enc_res=enc_res, encTs=encTs):
                    enc_f32 = stream.tile([P, D], F32, tag="enc_f32")
                    nc.sync.dma_start(
                        out=enc_f32, in_=enc[b, t * P:(t + 1) * P, :]
                    )
                    er = encresp.tile([P, D], BF16, tag="enc_res")
                    nc.vector.tensor_copy(er, enc_f32)
                    enc_res.append(er)

                    # transpose 8 [128,128] blocks via PE
                    encT = enctp.tile([P, DC, P], BF16, tag="encT")
                    for half in range(2):
                        pt = psT.tile([P, EH], BF16, tag="psT")
                        for j in range(4):
                            dcj = half * 4 + j
                            nc.tensor.transpose(
                                pt[:, j * P:(j + 1) * P],
                                er[:, dcj * P:(dcj + 1) * P],
                                ident,
                            )
                        nc.scalar.copy(
                            out=encT[:, half * 4:(half + 1) * 4, :],
                            in_=pt.rearrange("p (a b) -> p a b", a=4),
                        )
                    encTs[t] = encT

                stage_tile(0)
                stage_tile(1)
                for t in range(S_TILES):
                    if t + 2 < S_TILES:
                        stage_tile(t + 2)
                    encT = encTs.pop(t)

                    feats = featsp.tile([P, D], F32, tag="feats")
                    for eh in range(2):
                        pf = psF.tile([P, EH], F32, tag="psF")
                        for dc in range(DC):
                            nc.tensor.matmul(
                                pf,
                                lhsT=encT[:, dc, :],
                                rhs=W_hT[:, dc, eh * EH:(eh + 1) * EH],
                                start=(dc == 0),
                                stop=False,
                            )
                        # rank-2 last: cov[s]*w_c[e] + 1*dec_feat[e]
                        nc.tensor.matmul(
                            pf,
                            lhsT=cov_ones[:, t * P:(t + 1) * P],
                            rhs=wc_dec[:, eh * EH:(eh + 1) * EH],
                            start=False,
                            stop=True,
                        )
                        nc.scalar.activation(
                            feats[:, eh * EH:(eh + 1) * EH], pf, AF.Tanh
                        )

                    mv = mvp.tile([P, D], F32, tag="mv")
                    nc.gpsimd.tensor_tensor(mv, feats, v_rep, ALU.mult)
                    nc.vector.reduce_sum(
                        scores[:, t:t + 1], mv, axis=mybir.AxisListType.X
                    )

                # ---------------- phase B: softmax + context ----------------
                m1 = small.tile([P, 1], F32, tag="m1")
                nc.vector.reduce_max(m1, scores, axis=mybir.AxisListType.X)
                m_all = small.tile([P, 1], F32, tag="m_all")
                nc.gpsimd.partition_all_reduce(
                    m_all, m1, channels=P, reduce_op=bass_isa.ReduceOp.max
                )
                nm = small.tile([P, 1], F32, tag="nm")
                nc.vector.tensor_scalar_mul(nm, m_all, -1.0)

                p_exp = perb.tile([P, S_TILES], F32, tag="p_exp")
                nc.scalar.activation(p_exp, scores, AF.Exp, bias=nm, scale=1.0)

                q = perb.tile([P, S_TILES], F32, tag="q")
                nc.vector.tensor_tensor(q, p_exp, mask_pt, ALU.mult)
                z1 = small.tile([P, 1], F32, tag="z1")
                nc.vector.reduce_sum(z1, q, axis=mybir.AxisListType.X)
                z_all = small.tile([P, 1], F32, tag="z_all")
                nc.gpsimd.partition_all_reduce(
                    z_all, z1, channels=P, reduce_op=bass_isa.ReduceOp.add
                )
                rz = small.tile([P, 1], F32, tag="rz")
                nc.vector.reciprocal(rz, z_all)

                attn = perb.tile([P, S_TILES], F32, tag="attn")
                nc.vector.tensor_scalar_mul(attn, q, rz)
                # context uses unnormalized q (bf16); 1/Z is folded into the
                # final PSUM->SBUF copy so the ctx matmuls don't wait on the
                # z-reduce chain.
                q_bf = perb.tile([P, S_TILES], BF16, tag="attn_bf")
                nc.vector.tensor_copy(q_bf, q)

                cov_new = perb.tile([P, S_TILES], F32, tag="cov_new")
                nc.vector.tensor_tensor(cov_new, cov_pt, attn, ALU.add)

                nc.sync.dma_start(
                    out=attn_o[b, :].rearrange("(t p) -> p t", p=P), in_=attn
                )
                nc.sync.dma_start(
                    out=cov_o[b, :].rearrange("(t p) -> p t", p=P), in_=cov_new
                )

                # context = sum_s attn[s] * enc[s, :]
                # NB: keep each PSUM accumulation group's matmuls contiguous
                # (dc outer) — interleaving groups within a bank gives wrong
                # results on HW.
                pc = psC.tile([P, DC], F32, tag="psC")
                for dc in range(DC):
                    for t in range(S_TILES):
                        nc.tensor.matmul(
                            pc[:, dc:dc + 1],
                            lhsT=enc_res[t][:, dc * P:(dc + 1) * P],
                            rhs=q_bf[:, t:t + 1],
                            start=(t == 0),
                            stop=(t == S_TILES - 1),
                        )
                ctx_sb = small.tile([P, DC], F32, tag="ctx_sb")
                nc.vector.tensor_scalar_mul(ctx_sb, pc, rz)
                nc.sync.dma_start(
                    out=ctx_o[b, :].rearrange("(dc p) -> p dc", p=P), in_=ctx_sb
                )

    nc.compile()
    return nc


def kernel(encoder_output, decoder_hidden, mask, coverage, W_h, W_s, b_s, w_c, v):
    if "nc" not in _CACHE:
        _CACHE["nc"] = _build()
    nc = _CACHE["nc"]

    f = lambda x: np.ascontiguousarray(np.asarray(x, dtype=np.float32))
    enc, dec = f(encoder_output), f(decoder_hidden)
    msk, cov = f(mask), f(coverage)
    Wh, Ws, bs, wc, vv = f(W_h), f(W_s), f(b_s), f(w_c), f(v)

    in_maps = []
    for i in range(N_CORES):
        sl = slice(i * BPC, (i + 1) * BPC)
        in_maps.append({
            "encoder_output": enc[sl],
            "decoder_hidden": dec[sl],
            "mask": msk[sl],
            "coverage": cov[sl],
            "W_h": Wh,
            "W_s": Ws,
            "b_s": bs,
            "w_c": wc,
            "v": vv,
        })

    trace = bool(os.environ.get("KERNEL_TRACE"))
    res = run_bass_kernel_spmd(nc, in_maps, core_ids=list(range(N_CORES)),
                               trace=trace)
    _CACHE["last_results"] = res

    context = np.concatenate([res.results[i]["context_out"] for i in range(N_CORES)])
    attn = np.concatenate([res.results[i]["attn_out"] for i in range(N_CORES)])
    cov_new = np.concatenate([res.results[i]["coverage_out"] for i in range(N_CORES)])
    return context, attn, cov_new


# revision 33
# speedup vs baseline: 1.0012x; 1.0012x over previous
"""Bahdanau attention with coverage — Trainium2 Bass kernel, 8-core data parallel.

Shards batch B=16 across 8 NeuronCores (2 batches/core). Per core:
  enc_feat = enc @ W_h.T  (bf16 matmul, PE; enc tiles transposed on PE)
  + dec_feat + cov*w_c    (folded into the PSUM accumulation as a K=2 rank-2 matmul)
  tanh (ACT, reads PSUM) -> dot with v (gpsimd mult + DVE reduce) -> scores
  softmax over S=4096 (DVE free-reduce + gpsimd partition_all_reduce), mask+renorm
  context = attn @ enc    (PE, from resident bf16 copy of enc in SBUF)
"""

import os
import sys

import numpy as np

try:
    import concourse.bass as bass  # noqa: F401
except ImportError:
    sys.path.insert(0, "/opt/trn_rl_repo")

import concourse.bass as bass
import concourse.mybir as mybir
import concourse.tile as tile
from concourse import bacc, bass_isa
from concourse.bass_utils import run_bass_kernel_spmd
from concourse.masks import make_identity

P = 128
B = 16
S = 4096
D = 1024
N_CORES = 8
BPC = B // N_CORES        # batches per core
S_TILES = S // P          # 32
DC = D // P               # 8 d-chunks
EH = 512                  # psum half of e dim
F32 = mybir.dt.float32
BF16 = mybir.dt.bfloat16

_CACHE = {}


def _bcast_ap(dram_ap, n_part):
    """AP that replicates a 1-D dram tensor across n_part partitions."""
    return bass.AP(
        tensor=dram_ap.tensor,
        offset=dram_ap.offset,
        ap=[[0, n_part]] + list(dram_ap.ap),
    )


def _build():
    nc = bacc.Bacc("TRN2", target_bir_lowering=False, debug=False,
                   num_devices=N_CORES)

    enc = nc.dram_tensor("encoder_output", [BPC, S, D], F32, kind="ExternalInput")
    dec = nc.dram_tensor("decoder_hidden", [BPC, 1, D], F32, kind="ExternalInput")
    mask = nc.dram_tensor("mask", [BPC, S], F32, kind="ExternalInput")
    cov = nc.dram_tensor("coverage", [BPC, S], F32, kind="ExternalInput")
    W_h = nc.dram_tensor("W_h", [D, D], F32, kind="ExternalInput")
    W_s = nc.dram_tensor("W_s", [D, D], F32, kind="ExternalInput")
    b_s = nc.dram_tensor("b_s", [D], F32, kind="ExternalInput")
    w_c = nc.dram_tensor("w_c", [D], F32, kind="ExternalInput")
    v = nc.dram_tensor("v", [D], F32, kind="ExternalInput")
    ctx_o = nc.dram_tensor("context_out", [BPC, D], F32, kind="ExternalOutput")
    attn_o = nc.dram_tensor("attn_out", [BPC, S], F32, kind="ExternalOutput")
    cov_o = nc.dram_tensor("coverage_out", [BPC, S], F32, kind="ExternalOutput")

    AF = mybir.ActivationFunctionType
    ALU = mybir.AluOpType

    with tile.TileContext(nc) as tc:
        with (
            tc.tile_pool(name="singles", bufs=1) as singles,
            tc.tile_pool(name="wt", bufs=1) as wtp,
            tc.tile_pool(name="stream", bufs=4) as stream,
            tc.tile_pool(name="encres", bufs=38) as encresp,
            tc.tile_pool(name="enct", bufs=4) as enctp,
            tc.tile_pool(name="feats", bufs=3) as featsp,
            tc.tile_pool(name="mv", bufs=3) as mvp,
            tc.tile_pool(name="perb", bufs=2) as perb,
            tc.tile_pool(name="small", bufs=2) as small,
            tc.tile_pool(name="psT", bufs=3, space="PSUM") as psT,
            tc.tile_pool(name="psF", bufs=4, space="PSUM") as psF,
            tc.tile_pool(name="psC", bufs=1, space="PSUM") as psC,
        ):
            # ---------------- one-time constants ----------------
            # (small constant loads go on the gpsimd SWDGE queue so the sync
            # HWDGE queue starts on W_h immediately)
            ident = singles.tile([P, P], BF16)
            make_identity(nc, ident)

            v_rep = singles.tile([P, D], F32)
            nc.gpsimd.dma_start(out=v_rep, in_=_bcast_ap(v[:], P))

            b_s_sb = singles.tile([1, D], F32)
            nc.gpsimd.dma_start(out=b_s_sb, in_=b_s[None, :])

            w_c_bf = singles.tile([1, D], BF16)
            nc.gpsimd.dma_start(out=w_c_bf, in_=w_c[None, :])

            # Transposed bf16 weights: WT[di, dc, e] = W[e, dc*128+di]
            def load_transposed(Wdram, tag):
                WT = wtp.tile([P, DC, D], BF16, tag=tag)
                for i in range(DC):  # e-tile
                    wrow = stream.tile([P, D], F32, tag="enc_f32")
                    nc.sync.dma_start(out=wrow, in_=Wdram[i * P:(i + 1) * P, :])
                    wbf = enctp.tile([P, D], BF16, tag="encT")
                    nc.vector.tensor_copy(wbf, wrow)
                    for half in range(2):
                        pt = psT.tile([P, EH], BF16, tag="psT")
                        for j in range(4):
                            dcj = half * 4 + j
                            nc.tensor.transpose(
                                pt[:, j * P:(j + 1) * P],
                                wbf[:, dcj * P:(dcj + 1) * P],
                                ident,
                            )
                        nc.scalar.copy(
                            out=WT[:, half * 4:(half + 1) * 4, i * P:(i + 1) * P],
                            in_=pt.rearrange("p (a b) -> p a b", a=4),
                        )
                return WT

            # W_hT first: phase A's main matmuls depend only on it.
            W_hT = load_transposed(W_h, "W_hT")
            W_sT = load_transposed(W_s, "W_sT")

            # ---- dec rows for both batches (W_sT is transient) ----
            wc_decs = []
            for b in range(BPC):
                dec_sb = small.tile([P, DC], F32, tag="dec_sb")
                nc.sync.dma_start(
                    out=dec_sb, in_=dec[b, 0, :].rearrange("(do di) -> di do", di=P)
                )
                dec_bf = small.tile([P, DC], BF16, tag="dec_bf")
                nc.vector.tensor_copy(dec_bf, dec_sb)

                dec_row_f32 = small.tile([1, D], F32, tag="dec_row")
                for eh in range(2):
                    pd = psF.tile([1, EH], F32, tag="psF")
                    for dc in range(DC):
                        nc.tensor.matmul(
                            pd,
                            lhsT=dec_bf[:, dc:dc + 1],
                            rhs=W_sT[:, dc, eh * EH:(eh + 1) * EH],
                            start=(dc == 0),
                            stop=(dc == DC - 1),
                        )
                    nc.vector.tensor_tensor(
                        dec_row_f32[:, eh * EH:(eh + 1) * EH],
                        pd,
                        b_s_sb[:, eh * EH:(eh + 1) * EH],
                        ALU.add,
                    )
                dec_row_bf = small.tile([1, D], BF16, tag="dec_row_bf")
                nc.vector.tensor_copy(dec_row_bf, dec_row_f32)

                # wc_dec: [2, D] bf16 — row0 = w_c, row1 = dec_feat+b_s
                wc_dec = perb.tile([2, D], BF16, tag="wc_dec")
                nc.sync.dma_start(out=wc_dec[0:1, :], in_=w_c_bf)
                nc.sync.dma_start(out=wc_dec[1:2, :], in_=dec_row_bf)
                wc_decs.append(wc_dec)

            # cov_ones: [2, S] bf16 — row0 = coverage (f32->bf16 via gpsimd
            # converting DMA, per batch), row1 = ones (memset once).
            cov_ones = singles.tile([2, S], BF16, tag="cov_ones")
            nc.gpsimd.memset(cov_ones, 1.0)

            for b in range(BPC):
                wc_dec = wc_decs[b]

                nc.gpsimd.dma_start(out=cov_ones[0:1, :], in_=cov[b, None, :])

                # coverage & mask in [p, t] layout for the small phase
                cov_pt = perb.tile([P, S_TILES], F32, tag="cov_pt")
                nc.sync.dma_start(
                    out=cov_pt, in_=cov[b, :].rearrange("(t p) -> p t", p=P)
                )
                mask_pt = perb.tile([P, S_TILES], F32, tag="mask_pt")
                nc.sync.dma_start(
                    out=mask_pt, in_=mask[b, :].rearrange("(t p) -> p t", p=P)
                )

                scores = perb.tile([P, S_TILES], F32, tag="scores")

                # ---------------- phase A: scores ----------------
                # Software-pipelined: tile t+1's load/cast/transpose is issued
                # before tile t's matmuls so PE never stalls on the
                # transpose -> ACT-copyback -> matmul chain.
                enc_res = []
                encTs = {}

                def stage_tile(t, b=b, enc_res=enc_res, encTs=encTs):
                    enc_f32 = stream.tile([P, D], F32, tag="enc_f32")
                    nc.sync.dma_start(
                        out=enc_f32, in_=enc[b, t * P:(t + 1) * P, :]
                    )
                    er = encresp.tile([P, D], BF16, tag="enc_res")
                    nc.vector.tensor_copy(er, enc_f32)
                    enc_res.append(er)

                    # transpose 8 [128,128] blocks via PE
                    encT = enctp.tile([P, DC, P], BF16, tag="encT")
                    for half in range(2):
                        pt = psT.tile([P, EH], BF16, tag="psT")
                        for j in range(4):
                            dcj = half * 4 + j
                            nc.tensor.transpose(
                                pt[:, j * P:(j + 1) * P],
                                er[:, dcj * P:(dcj + 1) * P],
                                ident,
                            )
                        nc.scalar.copy(
                            out=encT[:, half * 4:(half + 1) * 4, :],
                            in_=pt.rearrange("p (a b) -> p a b", a=4),
                        )
                    encTs[t] = encT

                stage_tile(0)
                stage_tile(1)
                for t in range(S_TILES):
                    if t + 2 < S_TILES:
                        stage_tile(t + 2)
                    encT = encTs.pop(t)

                    feats = featsp.tile([P, D], F32, tag="feats")
                    for eh in range(2):
                        pf = psF.tile([P, EH], F32, tag="psF")
                        for dc in range(DC):
                            nc.tensor.matmul(
                                pf,
                                lhsT=encT[:, dc, :],
                                rhs=W_hT[:, dc, eh * EH:(eh + 1) * EH],
                                start=(dc == 0),
                                stop=False,
                            )
                        # rank-2 last: cov[s]*w_c[e] + 1*dec_feat[e]
                        nc.tensor.matmul(
                            pf,
                            lhsT=cov_ones[:, t * P:(t + 1) * P],
                            rhs=wc_dec[:, eh * EH:(eh + 1) * EH],
                            start=False,
                            stop=True,
                        )
                        nc.scalar.activation(
                            feats[:, eh * EH:(eh + 1) * EH], pf, AF.Tanh
                        )

                    mv = mvp.tile([P, D], F32, tag="mv")
                    nc.gpsimd.tensor_tensor(mv, feats, v_rep, ALU.mult)
                    nc.vector.reduce_sum(
                        scores[:, t:t + 1], mv, axis=mybir.AxisListType.X
                    )

                    if t == 27:
                        # softmax max is only a range shift (attn = q/sum(q)
                        # is exact for any m), so compute it from the first
                        # 28 tiles and overlap the reduce/allreduce chain
                        # with the remaining tiles' matmuls.
                        m1 = small.tile([P, 1], F32, tag="m1")
                        nc.vector.reduce_max(
                            m1, scores[:, :28], axis=mybir.AxisListType.X
                        )
                        m_all = small.tile([P, 1], F32, tag="m_all")
                        nc.gpsimd.partition_all_reduce(
                            m_all, m1, channels=P,
                            reduce_op=bass_isa.ReduceOp.max,
                        )
                        nm = small.tile([P, 1], F32, tag="nm")
                        nc.vector.tensor_scalar_mul(nm, m_all, -1.0)

                # ---------------- phase B: softmax + context ----------------
                p_exp = perb.tile([P, S_TILES], F32, tag="p_exp")
                nc.scalar.activation(p_exp, scores, AF.Exp, bias=nm, scale=1.0)

                q = perb.tile([P, S_TILES], F32, tag="q")
                nc.vector.tensor_tensor(q, p_exp, mask_pt, ALU.mult)
                z1 = small.tile([P, 1], F32, tag="z1")
                nc.vector.reduce_sum(z1, q, axis=mybir.AxisListType.X)
                z_all = small.tile([P, 1], F32, tag="z_all")
                nc.gpsimd.partition_all_reduce(
                    z_all, z1, channels=P, reduce_op=bass_isa.ReduceOp.add
                )
                rz = small.tile([P, 1], F32, tag="rz")
                nc.vector.reciprocal(rz, z_all)

                attn = perb.tile([P, S_TILES], F32, tag="attn")
                nc.vector.tensor_scalar_mul(attn, q, rz)
                # context uses unnormalized q (bf16); 1/Z is folded into the
                # final PSUM->SBUF copy so the ctx matmuls don't wait on the
                # z-reduce chain.
                q_bf = perb.tile([P, S_TILES], BF16, tag="attn_bf")
                nc.vector.tensor_copy(q_bf, q)

                cov_new = perb.tile([P, S_TILES], F32, tag="cov_new")
                nc.vector.tensor_tensor(cov_new, cov_pt, attn, ALU.add)

                nc.sync.dma_start(
                    out=attn_o[b, :].rearrange("(t p) -> p t", p=P), in_=attn
                )
                nc.sync.dma_start(
                    out=cov_o[b, :].rearrange("(t p) -> p t", p=P), in_=cov_new
                )

                # context = sum_s attn[s] * enc[s, :]
                # NB: keep each PSUM accumulation group's matmuls contiguous
                # (dc outer) — interleaving groups within a bank gives wrong
                # results on HW.
                pc = psC.tile([P, DC], F32, tag="psC")
                for dc in range(DC):
                    for t in range(S_TILES):
                        nc.tensor.matmul(
                            pc[:, dc:dc + 1],
                            lhsT=enc_res[t][:, dc * P:(dc + 1) * P],
                            rhs=q_bf[:, t:t + 1],
                            start=(t == 0),
                            stop=(t == S_TILES - 1),
                        )
                ctx_sb = small.tile([P, DC], F32, tag="ctx_sb")
                nc.vector.tensor_scalar_mul(ctx_sb, pc, rz)
                nc.sync.dma_start(
                    out=ctx_o[b, :].rearrange("(dc p) -> p dc", p=P), in_=ctx_sb
                )

    nc.compile()
    return nc


def kernel(encoder_output, decoder_hidden, mask, coverage, W_h, W_s, b_s, w_c, v):
    if "nc" not in _CACHE:
        _CACHE["nc"] = _build()
    nc = _CACHE["nc"]

    f = lambda x: np.ascontiguousarray(np.asarray(x, dtype=np.float32))
    enc, dec = f(encoder_output), f(decoder_hidden)
    msk, cov = f(mask), f(coverage)
    Wh, Ws, bs, wc, vv = f(W_h), f(W_s), f(b_s), f(w_c), f(v)

    in_maps = []
    for i in range(N_CORES):
        sl = slice(i * BPC, (i + 1) * BPC)
        in_maps.append({
            "encoder_output": enc[sl],
            "decoder_hidden": dec[sl],
            "mask": msk[sl],
            "coverage": cov[sl],
            "W_h": Wh,
            "W_s": Ws,
            "b_s": bs,
            "w_c": wc,
            "v": vv,
        })

    trace = bool(os.environ.get("KERNEL_TRACE"))
    res = run_bass_kernel_spmd(nc, in_maps, core_ids=list(range(N_CORES)),
                               trace=trace)
    _CACHE["last_results"] = res

    context = np.concatenate([res.results[i]["context_out"] for i in range(N_CORES)])
    attn = np.concatenate([res.results[i]["attn_out"] for i in range(N_CORES)])
    cov_new = np.concatenate([res.results[i]["coverage_out"] for i in range(N_CORES)])
    return context, attn, cov_new
